# revision 12
# baseline (speedup 1.0000x reference)
"""Trainium2 Bass kernel: 4096x4096 fp32 image, 9x9 valid cross-correlation + bias.

Strategy
--------
Shard the image across 8 NeuronCores on an (RG x CG) grid (RG row groups x
CG column groups, kernel/bias replicated; each core gets its stripe plus an
8-row/8-col halo, so no collectives are needed).

Per core the conv runs on the tensor engine as banded matmuls:

  psum[m, n] = sum_dj sum_k B_dj[k, m] * X[r0+k, c0+dj+n]

where B_dj[k, m] = kern[k-m, dj] for 0 <= k-m < 9 (else 0) is a 128x120
banded Toeplitz stationary operand built on the host from the 9x9 kernel.
One PSUM accumulation group of 9 matmuls (one per kernel column dj, with rhs
= plain column-offset views of the same SBUF tile) covers all 81 taps of a
[120 out-rows x N out-cols] tile.

Operands are bf16 (X/B quantized on host): the PE runs bf16 at 1 cycle/row
vs fp32's 4, and input/output DMA bytes halve; accumulation stays fp32 in
PSUM, bias-add on DVE, output stored bf16 and upcast on the host (randn
inputs, 81-tap sums: quantization rel-err ~4e-3, well under the 2e-2 gate).

All input-block DMAs are issued up front (the stripe fits in SBUF) so no
matmul ever waits on a load; the PSUM->SBUF move is fused with the bias add
in a single DVE tensor_scalar op per block, and per-block output DMAs
pipeline behind it.

`repeat`/`hwloop` build timing variants: `hwloop=True` wraps the body in a
hardware For_i loop so the program size stays constant while the body runs
`repeat` times -- the (T(R2)-T(R1))/(R2-R1) delta then measures pure
execution of one conv pass, uncontaminated by NEFF-load time.
"""

import numpy as np
import ml_dtypes

H, W = 4096, 4096
KH, KW = 9, 9
NCORES = 8
OH, OW = H - KH + 1, W - KW + 1  # 4088, 4088
MB = 120  # output rows per full row block (128 input rows - 8)

GRID = (2, 4)  # (row groups, col groups), RG*CG == 8
DTYPE = "bf16"  # "bf16" | "f32r" | "f32"


def _geom(grid):
    RG, CG = grid
    assert RG * CG == NCORES
    RPC = OH // RG  # out rows per core
    CPC = OW // CG  # out cols per core
    IN_ROWS = RPC + KH - 1
    IN_COLS = CPC + KW - 1
    nfull = RPC // MB
    tail = RPC - nfull * MB
    blocks = [(b * MB, 128, MB) for b in range(nfull)]
    if tail:
        blocks.append((nfull * MB, tail + KH - 1, tail))
    return RPC, CPC, IN_ROWS, IN_COLS, blocks


def _build_nc(repeat=1, dtype=DTYPE, hwloop=False, grid=GRID, unroll=1):
    import concourse.bacc as bacc
    import concourse.mybir as mybir
    import concourse.tile as tile

    RPC, CPC, IN_ROWS, IN_COLS, blocks = _geom(grid)

    F32 = mybir.dt.float32
    # f32r: tensors/DMA stay plain fp32 (PJRT can't bind float32r buffers);
    # only the matmul operand APs are bitcast to float32r, which selects the
    # PE's fast single-pass fp32 mode -- unusable here (walrus crashes) but
    # kept for reference.
    DT = {
        "bf16": mybir.dt.bfloat16,
        "bf16f": mybir.dt.bfloat16,  # bf16 compute, fp32 output store
        "f32r": F32,
        "f32": F32,
    }[dtype]
    ODT = mybir.dt.bfloat16 if dtype == "bf16" else F32

    def mm_cast(ap):
        return ap.bitcast(mybir.dt.float32r) if dtype == "f32r" else ap

    nc = bacc.Bacc("TRN2", target_bir_lowering=False, debug=False)
    Xs = nc.dram_tensor("Xs", [IN_ROWS, IN_COLS], DT, kind="ExternalInput")
    Bm = nc.dram_tensor("Bm", [128, KW * MB], DT, kind="ExternalInput")
    Bc = nc.dram_tensor("Bc", [128, 1], F32, kind="ExternalInput")
    O = nc.dram_tensor("O", [RPC, CPC], ODT, kind="ExternalOutput")

    # matmul output must stay inside one PSUM bank (512 fp32): split the
    # core's CPC output columns into <=511-wide column tiles
    csplits = [(c0, min(511, CPC - c0)) for c0 in range(0, CPC, 511)]
    pp_bufs = 8

    with tile.TileContext(nc) as tc:
        with (
            tc.tile_pool(name="const", bufs=1) as cpool,
            tc.tile_pool(name="xp", bufs=len(blocks)) as xp,
            tc.tile_pool(name="op", bufs=3) as op,
            tc.tile_pool(name="pp", bufs=pp_bufs, space="PSUM") as pp,
        ):
            b_sb = cpool.tile([128, KW * MB], DT)
            nc.sync.dma_start(b_sb[:], Bm[:])
            bias_sb = cpool.tile([128, 1], F32)
            nc.sync.dma_start(bias_sb[:], Bc[:])

            def body():
                xts = []
                for r0, kb, mb in blocks:
                    xt = xp.tile([128, IN_COLS], DT, tag="x")
                    nc.sync.dma_start(xt[:kb, :], Xs[r0 : r0 + kb, :])
                    xts.append(xt)
                for (r0, kb, mb), xt in zip(blocks, xts):
                    ot = op.tile([128, CPC], ODT, tag="o")
                    for c0, cw in csplits:
                        ps = pp.tile([128, 511], F32, tag="ps")
                        for dj in range(KW):
                            nc.tensor.matmul(
                                ps[:mb, :cw],
                                mm_cast(b_sb[:kb, dj * MB : dj * MB + mb]),
                                mm_cast(xt[:kb, c0 + dj : c0 + dj + cw]),
                                start=(dj == 0),
                                stop=(dj == KW - 1),
                            )
                        nc.vector.tensor_scalar_add(
                            ot[:mb, c0 : c0 + cw], ps[:mb, :cw], bias_sb[:mb, 0:1]
                        )
                    nc.sync.dma_start(O[r0 : r0 + mb, :], ot[:mb, :])

            if hwloop:
                assert repeat % unroll == 0
                with tc.For_i(0, repeat // unroll):
                    for _ in range(unroll):
                        body()
            else:
                for _ in range(repeat):
                    body()

    nc.compile()
    return nc


def _np_dt(dtype):
    return ml_dtypes.bfloat16 if dtype in ("bf16", "bf16f") else np.float32


def _host_inputs(X, kern, bias, dtype=DTYPE, grid=GRID):
    """Per-core input maps: grid-sharded X with halo + replicated band/bias."""
    RG, CG = grid
    RPC, CPC, IN_ROWS, IN_COLS, _ = _geom(grid)
    ndt = _np_dt(dtype)
    X = np.ascontiguousarray(np.asarray(X, dtype=np.float32)).astype(ndt)
    kern = np.asarray(kern, dtype=np.float32)
    bias = np.asarray(bias, dtype=np.float32)

    Bm = np.zeros((128, KW * MB), np.float32)
    m = np.arange(MB)
    for dj in range(KW):
        for d in range(KH):
            Bm[m + d, dj * MB + m] = kern[d, dj]
    Bm = Bm.astype(ndt)
    Bc = np.full((128, 1), bias[0], np.float32)

    maps = []
    for c in range(NCORES):
        rg, cg = divmod(c, CG)
        maps.append(
            {
                "Xs": np.ascontiguousarray(
                    X[
                        rg * RPC : rg * RPC + IN_ROWS,
                        cg * CPC : cg * CPC + IN_COLS,
                    ]
                ),
                "Bm": Bm,
                "Bc": Bc,
            }
        )
    return maps


_NC_CACHE = {}


def _get_nc(repeat=1, dtype=DTYPE, hwloop=False, grid=GRID, unroll=1):
    key = (repeat, dtype, hwloop, grid, unroll)
    if key not in _NC_CACHE:
        _NC_CACHE[key] = _build_nc(repeat, dtype, hwloop, grid, unroll)
    return _NC_CACHE[key]


def kernel(X, kernel, bias):
    from concourse.bass_utils import run_bass_kernel_spmd

    RG, CG = GRID
    RPC, CPC, _, _, _ = _geom(GRID)
    nc = _get_nc()
    in_maps = _host_inputs(X, kernel, bias)
    res = run_bass_kernel_spmd(nc, in_maps, core_ids=list(range(NCORES)))
    out = np.empty((OH, OW), np.float32)
    for c in range(NCORES):
        rg, cg = divmod(c, CG)
        out[rg * RPC : (rg + 1) * RPC, cg * CPC : (cg + 1) * CPC] = res.results[c][
            "O"
        ].astype(np.float32)
    return out
